# revision 1
# baseline (speedup 1.0000x reference)
# MoE routing + sparse-frequency inverse FFT2 kernel for Trainium2 (8 NeuronCores).
#
# Math: out_b = ALPHA * Re(ifft2(mask_b)) where mask_b has 4096 nonzero
# frequencies (top-2 experts x 2048 each).  With the symmetric real DFT basis
#   C[x,u] = cos(2*pi*x*u/768)/768,  S[x,u] = sin(2*pi*x*u/768)/768
# the dense iFFT2 factorizes into four 768^3 matmuls per sample:
#   out = (C @ (M @ C) - S @ (M @ S)) * ALPHA
# Device work per core (4 samples): router GEMM, top-2 selection and weights,
# per-expert entry gather (row-granular indirect DMA at offsets computed from
# the one-hot expert selection), sparse->dense mask build via iota/compare
# one-hots placed with PE matmuls, then the four big matmuls in float32r.
# Host only prepares input-layout constants: the C/S tables and a bucketed,
# padded, partition-major re-layout of the static (list_indices, coeff)
# tables, plus batch sharding.
#
# Element-granular DMA scatter is avoided on purpose: TRN2's indirect DMA is
# row-granular (one offset per partition, contiguous run per partition), so
# the mask is built from gathered (u, v, val) entry groups instead.

import sys

sys.path.insert(0, "/opt/trn_rl_repo")

import numpy as np

import concourse.bacc as bacc
import concourse.mybir as mybir
import concourse.tile as tile
from concourse.bass import IndirectOffsetOnAxis
from concourse.bass_utils import run_bass_kernel_spmd
from concourse.masks import make_identity

N = 768
E = 64
NF = 2048
B = 32
NCORES = 8
BPC = B // NCORES          # samples per core
NBLK = 6                   # 768 / 128
ALPHA = 300.0
GRID = N * N
HALF = N // 2 + 2          # 386 computed stage-1 columns (even width for f32r)

# per-(expert, v-chunk) buckets, sub-bucketed by u-range so each build matmul
# is one bank-aligned instruction: u in [0,512) padded to 384, u in [512,768)
# padded to 256.  Pads sit >=11 sigma above the expected bucket fills.
HB = ((0, 512, 384), (512, 256, 256))   # (u0, uwidth, pad)
BROW = sum(p for _, _, p in HB)          # 640 entries per (expert, v-chunk)
EROW = NBLK * BROW                       # 3840 entries per expert
COLS = EROW // 128                       # 30 gather columns per expert slot

F32 = mybir.dt.float32
F32R = mybir.dt.float32r
I32 = mybir.dt.int32
AOT = mybir.AluOpType

KERNEL_TRACE = False       # test harness can flip this to profile
LAST_RESULT = None

_NC = None


def _build():
    nc = bacc.Bacc(trn_type="TRN2")

    cls4 = nc.dram_tensor("cls4", [BPC, N], F32, kind="ExternalInput")
    wr = nc.dram_tensor("wr", [E, N], F32, kind="ExternalInput")
    br = nc.dram_tensor("br", [E], F32, kind="ExternalInput")
    u2 = nc.dram_tensor("u2", [E, EROW], F32, kind="ExternalInput")
    vm2 = nc.dram_tensor("vm2", [E, EROW], F32, kind="ExternalInput")
    cv2 = nc.dram_tensor("cv2", [E, EROW], F32, kind="ExternalInput")
    bases = nc.dram_tensor("bases", [E, 1], F32, kind="ExternalInput")
    jm = nc.dram_tensor("jm", [128, 128], F32R, kind="ExternalInput")
    ct = nc.dram_tensor("ct", [N, N], F32R, kind="ExternalInput")
    st = nc.dram_tensor("st", [N, N], F32R, kind="ExternalInput")
    out4 = nc.dram_tensor("out4", [BPC, N, N], F32, kind="ExternalOutput")

    with tile.TileContext(nc) as tc:
        with (
            tc.tile_pool(name="const", bufs=1) as cpool,
            tc.tile_pool(name="tables", bufs=1) as tpool,
            tc.tile_pool(name="routing", bufs=1) as rpool,
            tc.tile_pool(name="gath", bufs=1) as gpool,
            tc.tile_pool(name="build", bufs=20) as bpool,
            tc.tile_pool(name="mt", bufs=2) as mtpool,
            tc.tile_pool(name="pq", bufs=1) as pqpool,
            tc.tile_pool(name="outp", bufs=3) as opool,
            tc.tile_pool(name="psA", bufs=2, space="PSUM") as psA,
            tc.tile_pool(name="psA1", bufs=3, space="PSUM") as psA1,
            tc.tile_pool(name="psB", bufs=1, space="PSUM") as psB,
            tc.tile_pool(name="mir", bufs=2) as mirpool,
            tc.tile_pool(name="t1p", bufs=2) as t1pool,
        ):
            ident = cpool.tile([128, 128], F32)
            make_identity(nc, ident[:])
            ones1 = cpool.tile([1, 128], F32)
            nc.vector.memset(ones1[:], 1.0)
            ones14 = cpool.tile([1, BPC], F32)
            nc.vector.memset(ones14[:], 1.0)
            i768 = cpool.tile([128, N], I32)
            nc.gpsimd.iota(i768[:], pattern=[[1, N]], base=0, channel_multiplier=0)
            i768f = cpool.tile([128, N], F32)
            nc.vector.tensor_copy(i768f[:], i768[:])
            i128f = cpool.tile([128, 128], F32)
            nc.vector.tensor_copy(i128f[:], i768[:, 0:128])
            io24 = cpool.tile([128, 1], I32)
            nc.gpsimd.iota(io24[:], pattern=[[0, 1]], base=0, channel_multiplier=COLS)
            io24f = cpool.tile([128, 1], F32)
            nc.vector.tensor_copy(io24f[:], io24[:])

            br_sb = rpool.tile([1, E], F32)
            nc.sync.dma_start(out=br_sb[:], in_=br[None, :])
            bases_sb = rpool.tile([E, 1], F32)
            nc.sync.dma_start(out=bases_sb[:], in_=bases[:])
            jJ = cpool.tile([128, 128], F32R)
            nc.sync.dma_start(out=jJ[:], in_=jm[:])

            # ---- router: logits = cls4 @ Wr.T + br ----
            comb = rpool.tile([E + BPC, N], F32)
            nc.sync.dma_start(out=comb[0:BPC, :], in_=cls4[:])
            nc.sync.dma_start(out=comb[BPC : BPC + E, :], in_=wr[:])
            combt = rpool.tile([128, NBLK * (E + BPC)], F32)
            for j in range(NBLK):
                tp = psB.tile([128, E + BPC], F32, tag="small")
                nc.tensor.transpose(
                    tp[:],
                    comb[0 : E + BPC, 128 * j : 128 * (j + 1)],
                    ident[0 : E + BPC, 0 : E + BPC],
                )
                nc.scalar.copy(combt[:, (E + BPC) * j : (E + BPC) * (j + 1)], tp[:])
            lg_ps = psB.tile([BPC, E], F32, tag="small")
            for j in range(NBLK):
                base = (E + BPC) * j
                nc.tensor.matmul(
                    lg_ps[:],
                    lhsT=combt[:, base : base + BPC],
                    rhs=combt[:, base + BPC : base + BPC + E],
                    start=(j == 0),
                    stop=False,
                )
            nc.tensor.matmul(
                lg_ps[:], lhsT=ones14[:], rhs=br_sb[:], start=False, stop=True
            )
            logits = rpool.tile([BPC, E], F32)
            nc.vector.tensor_copy(logits[:], lg_ps[:])

            # ---- top-2, renormalized weights, one-hot selectors ----
            max8 = rpool.tile([BPC, 8], F32)
            nc.vector.max(out=max8[:], in_=logits[:])
            l0 = max8[:, 0:1]
            l1 = max8[:, 1:2]
            d = rpool.tile([BPC, 1], F32)
            nc.vector.tensor_sub(d[:], l1, l0)  # l1 - l0
            dT_ps = psB.tile([1, BPC], F32, tag="small")
            nc.tensor.transpose(dT_ps[:], d[:], ident[0:BPC, 0:BPC])
            dT = rpool.tile([1, BPC], F32)
            nc.vector.tensor_copy(dT[:], dT_ps[:])
            w1T = rpool.tile([1, BPC], F32)
            nc.scalar.activation(w1T[:], dT[:], mybir.ActivationFunctionType.Sigmoid)
            w0T = rpool.tile([1, BPC], F32)
            nc.scalar.activation(
                w0T[:], dT[:], mybir.ActivationFunctionType.Sigmoid, scale=-1.0
            )
            oh1 = rpool.tile([BPC, E], F32)
            oh2 = rpool.tile([BPC, E], F32)
            nc.vector.tensor_scalar(oh1[:], logits[:], l0, None, op0=AOT.is_equal)
            nc.vector.tensor_scalar(oh2[:], logits[:], l1, None, op0=AOT.is_equal)
            selT = []
            for srcap in (oh1, oh2):
                sp = psB.tile([E, BPC], F32, tag="small")
                nc.tensor.transpose(sp[:], srcap[:], ident[0:BPC, 0:BPC])
                sbt = rpool.tile([E, BPC], F32, tag=f"selT{len(selT)}")
                nc.vector.tensor_copy(sbt[:], sp[:])
                selT.append(sbt)
            o1T, o2T = selT

            # per-sample scalar rows [1, BPC]: expert table offsets
            eT = []
            for oT in (o1T, o2T):
                ep = psB.tile([1, BPC], F32, tag="small")
                nc.tensor.matmul(
                    ep[:], lhsT=bases_sb[:], rhs=oT[:], start=True, stop=True
                )
                es = rpool.tile([1, BPC], F32, tag=f"eT{len(eT)}")
                nc.vector.tensor_copy(es[:], ep[:])
                eT.append(es)

            # broadcast each scalar row to all 128 partitions: [128, BPC]
            bc = []
            for rowap in (eT[0], eT[1], w0T, w1T):
                bp = psB.tile([128, BPC], F32, tag="small")
                nc.tensor.matmul(
                    bp[:], lhsT=ones1[:], rhs=rowap[:], start=True, stop=True
                )
                bs = rpool.tile([128, BPC], F32, tag=f"bc{len(bc)}")
                nc.vector.tensor_copy(bs[:], bp[:])
                bc.append(bs)
            ebc = bc[0:2]    # expert base offsets per slot
            wbc = bc[2:4]    # expert weights per slot

            # ---- C/S table loads AFTER the routing-phase emission so the
            # small router DMAs aren't queued behind 4.7 MB on the sync FIFO
            ct_sb = tpool.tile([128, NBLK * N], F32R, tag="ct")
            st_sb = tpool.tile([128, NBLK * N], F32R, tag="st")
            for j in range(NBLK):
                nc.sync.dma_start(
                    out=ct_sb[:, N * j : N * (j + 1)],
                    in_=ct[128 * j : 128 * (j + 1), :],
                )
                nc.sync.dma_start(
                    out=st_sb[:, N * j : N * (j + 1)],
                    in_=st[128 * j : 128 * (j + 1), :],
                )

            ych = [(0, 512), (512, 256)]

            def emit_d(b, di, mc):
                dps = psA.tile([128, N], F32, tag="mm")
                for c0, cw in ych:
                    nc.tensor.matmul(
                        dps[:, c0 : c0 + cw],
                        lhsT=jJ[:],
                        rhs=mc[:, c0 : c0 + cw],
                        start=True, stop=True,
                    )
                ob = opool.tile([128, N], F32, tag="ob")
                nc.scalar.copy(ob[:], dps[:])
                nc.scalar.dma_start(
                    out=out4[:][b][128 * (4 + di) : 128 * (5 + di), :], in_=ob[:]
                )

            # ---- gather ALL samples' (u, vm, coeff) entry tables upfront ----
            allg = []
            for b in range(BPC):
                gus, gvms, gcws = [], [], []
                for slot in range(2):
                    offf = gpool.tile([128, 1], F32, tag="offf")
                    nc.vector.tensor_add(offf[:], ebc[slot][:, b : b + 1], io24f[:])
                    offs = gpool.tile([128, 1], I32, tag="offs")
                    nc.vector.tensor_copy(offs[:], offf[:])
                    gu = gpool.tile([128, COLS], F32, tag=f"gu{b}_{slot}")
                    gvm = gpool.tile([128, COLS], F32, tag=f"gvm{b}_{slot}")
                    gcv = gpool.tile([128, COLS], F32, tag=f"gcv{b}_{slot}")
                    for tab, dst in ((u2, gu), (vm2, gvm), (cv2, gcv)):
                        nc.gpsimd.indirect_dma_start(
                            out=dst[:],
                            out_offset=None,
                            in_=tab[:],
                            in_offset=IndirectOffsetOnAxis(ap=offs[:], axis=1),
                        )
                    gcw = gpool.tile([128, COLS], F32, tag=f"gcw{b}_{slot}")
                    nc.vector.tensor_scalar(
                        gcw[:], gcv[:], wbc[slot][:, b : b + 1], None, op0=AOT.mult
                    )
                    gus.append(gu)
                    gvms.append(gvm)
                    gcws.append(gcw)
                allg.append((gus, gvms, gcws))

            for b in range(BPC):
                gus, gvms, gcws = allg[b]
                # ---- build MT (transposed mask) chunk by chunk on PE ----
                mt_sb = mtpool.tile([128, NBLK * N], F32R, tag="mt")
                for j in range(NBLK):
                    mtps = psA.tile([128, N], F32, tag="mm")
                    colbase = COLS * j // NBLK * 0  # columns laid out per j below
                    for hi, (u0, uw, pad) in enumerate(HB):
                        ng = pad // 128
                        coff = 5 * j + (0 if hi == 0 else HB[0][2] // 128)
                        for slot in range(2):
                            for g in range(ng):
                                col = coff + g
                                voh = bpool.tile([128, 128], F32R, tag="voh")
                                nc.vector.tensor_scalar(
                                    voh[:], i128f[:], gvms[slot][:, col : col + 1],
                                    None, op0=AOT.is_equal,
                                )
                                rhsb = bpool.tile([128, 512], F32R, tag="rhsb")
                                nc.vector.tensor_scalar(
                                    rhsb[:, 0:uw], i768f[:, u0 : u0 + uw],
                                    gus[slot][:, col : col + 1],
                                    gcws[slot][:, col : col + 1],
                                    op0=AOT.is_equal, op1=AOT.mult,
                                )
                                nc.tensor.matmul(
                                    mtps[:, u0 : u0 + uw],
                                    lhsT=voh[:],
                                    rhs=rhsb[:, 0:uw],
                                    start=(slot == 0 and g == 0),
                                    stop=(slot == 1 and g == ng - 1),
                                )
                    nc.scalar.copy(mt_sb[:, N * j : N * (j + 1)], mtps[:])

                # ---- stage 1 (paired): P = 300*(M @ C), Qn = -300*(M @ S) ----
                # only columns [0, HALF) are computed; C-column symmetry gives
                # P[:, N-y] = P[:, y] and Qn[:, N-y] = -Qn[:, y].
                p_sb = pqpool.tile([128, NBLK * N], F32R, tag="p")
                q_sb = pqpool.tile([128, NBLK * N], F32R, tag="q")
                for i in range(NBLK):
                    pps = psA1.tile([128, HALF], F32, tag="mm1")
                    qps = psA1.tile([128, HALF], F32, tag="mm1")
                    for k in range(NBLK):
                        lhs = mt_sb[:, N * k + 128 * i : N * k + 128 * (i + 1)]
                        nc.tensor.matmul(
                            pps[:], lhsT=lhs, rhs=ct_sb[:, N * k : N * k + HALF],
                            start=(k == 0), stop=(k == NBLK - 1),
                        )
                        nc.tensor.matmul(
                            qps[:], lhsT=lhs, rhs=st_sb[:, N * k : N * k + HALF],
                            start=(k == 0), stop=(k == NBLK - 1),
                        )
                    nc.scalar.mul(p_sb[:, N * i : N * i + HALF], pps[:], ALPHA)
                    nc.scalar.mul(q_sb[:, N * i : N * i + HALF], qps[:], -ALPHA)
                    nc.scalar.copy(
                        p_sb[:, N * i + HALF : N * (i + 1)],
                        p_sb[:][:, N * i + (N - HALF) : N * i : -1],
                    )
                    nc.scalar.mul(
                        q_sb[:, N * i + HALF : N * (i + 1)],
                        q_sb[:][:, N * i + (N - HALF) : N * i : -1],
                        -1.0,
                    )

                # ---- stage 2: rows 0..511 as T1+T2; rows 512..767 mirrored ----
                # T1 = C @ P, T2 = S @ Qn (both already x300).  Row symmetry:
                # out[N-x] = T1[x] - T2[x], realized with shifted anti-identity
                # matmuls (jA, jB) on M_i = T1_i - T2_i.
                mirs = []
                for i in range(4):
                    t1 = psA.tile([128, N], F32, tag="mm")
                    t2 = psA.tile([128, N], F32, tag="mm")
                    for dst, tbl, srcm in ((t1, ct_sb, p_sb), (t2, st_sb, q_sb)):
                        for k in range(NBLK):
                            for c0, cw in ych:
                                nc.tensor.matmul(
                                    dst[:, c0 : c0 + cw],
                                    lhsT=tbl[:, N * k + 128 * i : N * k + 128 * (i + 1)],
                                    rhs=srcm[:, N * k + c0 : N * k + c0 + cw],
                                    start=(k == 0),
                                    stop=(k == NBLK - 1),
                                )
                    t1s = t1pool.tile([128, N], F32, tag="t1")
                    nc.scalar.copy(t1s[:], t1[:])
                    ob = opool.tile([128, N], F32, tag="ob")
                    nc.vector.tensor_tensor(ob[:], t1s[:], t2[:], op=AOT.add)
                    nc.scalar.dma_start(
                        out=out4[:][b][128 * i : 128 * (i + 1), :], in_=ob[:]
                    )
                    # mirror source tiles: mc[d] rows = T1-T2 at x = (2-d)*128 - m
                    if i == 0:
                        m = mirpool.tile([128, N], F32R, tag="mc1")
                        nc.vector.tensor_tensor(m[:], t1s[:], t2[:], op=AOT.subtract)
                        mirs.append(m)  # mc2 body (block 0), row 0 patched later
                    elif i == 1:
                        m = mirpool.tile([128, N], F32R, tag="mc0")
                        nc.vector.tensor_tensor(m[:], t1s[:], t2[:], op=AOT.subtract)
                        mirs.append(m)  # mc1 body (block 1), row 0 patched later
                        nc.vector.tensor_tensor(
                            mirs[0][0:1, :], t1s[0:1, :], t2[0:1, :], op=AOT.subtract
                        )  # mc2 row 0 = block-1 row 0 (x = 128)
                    elif i == 2:
                        nc.vector.tensor_tensor(
                            mirs[1][0:1, :], t1s[0:1, :], t2[0:1, :], op=AOT.subtract
                        )  # mc1 row 0 = block-2 row 0 (x = 256)
                emit_d(b, 0, mirs[1])
                emit_d(b, 1, mirs[0])

    nc.compile()
    return nc


def _get_nc():
    global _NC
    if _NC is None:
        _NC = _build()
    return _NC


def _host_tables():
    a = np.arange(N, dtype=np.int64)
    ang = (2.0 * np.pi / N) * ((a[:, None] * a[None, :]) % N)
    ctv = (np.cos(ang) / N).astype(np.float32)
    stv = (np.sin(ang) / N).astype(np.float32)
    return ctv, stv


def _host_entry_tables(list_indices, coeff):
    """Bucket each expert's (u, v, coeff) entries by v-chunk, pad buckets to
    PAD, and lay out partition-major (entry 128*g + p lands at column g of
    partition p's contiguous gather run)."""
    li = list_indices.astype(np.int64)
    uu = li // N
    vv = li % N
    u2 = np.zeros((E, EROW), np.float32)
    vm2 = np.full((E, EROW), -9.0, np.float32)
    cv2 = np.zeros((E, EROW), np.float32)
    for e in range(E):
        for j in range(NBLK):
            selj = vv[e] // 128 == j
            base = BROW * j
            for u0, uw, pad in HB:
                sel = np.where(selj & (uu[e] >= u0) & (uu[e] < u0 + uw))[0]
                cnt = len(sel)
                assert cnt <= pad, f"bucket overflow: e{e} j{j} u{u0}: {cnt}"
                u2[e, base : base + cnt] = uu[e, sel]
                vm2[e, base : base + cnt] = vv[e, sel] - 128 * j
                cv2[e, base : base + cnt] = coeff[e, sel]
                base += pad
    # partition-major runs: table[e, p*COLS + g] = arr[e, 128*g + p]
    perm = np.array([128 * g + p for p in range(128) for g in range(COLS)])
    return u2[:, perm], vm2[:, perm], cv2[:, perm]


def kernel(cls_token, W_router, b_router, coeff, list_indices):
    global LAST_RESULT
    cls_token = np.asarray(cls_token)
    W_router = np.asarray(W_router)
    b_router = np.asarray(b_router)
    coeff = np.asarray(coeff)
    list_indices = np.asarray(list_indices)
    assert cls_token.shape == (B, N) and coeff.shape == (E, NF)
    nc = _get_nc()
    ctv, stv = _host_tables()
    u2v, vm2v, cv2v = _host_entry_tables(list_indices, coeff)
    basesv = (np.arange(E, dtype=np.float32) * EROW).reshape(E, 1)
    jmv = np.zeros((128, 128), np.float32)
    for m_ in range(128):
        jmv[(128 - m_) % 128, m_] = 1.0
    wrr = np.ascontiguousarray(W_router, dtype=np.float32)
    brr = np.ascontiguousarray(b_router, dtype=np.float32)
    in_maps = []
    for c in range(NCORES):
        in_maps.append(
            {
                "cls4": np.ascontiguousarray(
                    cls_token[BPC * c : BPC * (c + 1)], dtype=np.float32
                ),
                "wr": wrr,
                "br": brr,
                "u2": u2v,
                "vm2": vm2v,
                "cv2": cv2v,
                "bases": basesv,
                "jm": jmv,
                "ct": ctv,
                "st": stv,
            }
        )
    res = run_bass_kernel_spmd(
        nc, in_maps, core_ids=list(range(NCORES)), trace=KERNEL_TRACE
    )
    LAST_RESULT = res
    out = np.concatenate([res.results[c]["out4"] for c in range(NCORES)], axis=0)
    return out



# revision 6
# speedup vs baseline: 1.5324x; 1.5324x over previous
# MoE routing + sparse-frequency inverse FFT2 kernel for Trainium2 (8 NeuronCores).
#
# Math: out_b = ALPHA * Re(ifft2(mask_b)) where mask_b has 4096 nonzero
# frequencies (top-2 experts x 2048 each).  With the symmetric real DFT basis
#   C[x,u] = cos(2*pi*x*u/768)/768,  S[x,u] = sin(2*pi*x*u/768)/768
# the dense iFFT2 factorizes into per-sample matmuls:
#   P = ALPHA*(M @ C), Qn = -ALPHA*(M @ S);  T1 = C @ P, T2 = S @ Qn
#   out[:, y]   = (T1+T2)[:, y]            for y in [0, 386)
#   out[:, N-y] = (T1-T2)[:, y]            (column symmetry: C even, S odd)
#   out[N-x, :] mirrors via shifted anti-identity matmuls on (T1-T2 | rev(T1+T2))
# All heavy matmuls run in fp16 (1 cycle/row on PE vs ~1.5+overhead for f32r;
# integer indices <= 2048 are exact in fp16 so iota/compare one-hots stay
# exact).  Stage-1/2 compute only 386 of 768 columns; the rest is add/sub +
# reversed-stride copies.
# Device work per core (4 samples): router GEMM, top-2 selection and weights,
# per-expert entry gather (one fused row-granular indirect DMA per slot),
# sparse->dense mask build via iota/compare one-hots placed with PE matmuls,
# then the fp16 matmul pipeline above.  Host only prepares input-layout
# constants: fp16 C/S tables (512 cols), a bucketed, padded, partition-major,
# u/vm/cv-interleaved re-layout of the static (list_indices, coeff) tables,
# plus batch sharding.

import sys

sys.path.insert(0, "/opt/trn_rl_repo")

import numpy as np

import concourse.bacc as bacc
import concourse.mybir as mybir
import concourse.tile as tile
from concourse.bass import IndirectOffsetOnAxis
from concourse.bass_utils import run_bass_kernel_spmd
from concourse.masks import make_identity

N = 768
E = 64
NF = 2048
B = 32
NCORES = 8
BPC = B // NCORES          # samples per core
NBLK = 6                   # 768 / 128
ALPHA = 300.0
GRID = N * N
HALF = N // 2 + 2          # 386 computed stage-1/2 columns (even width)
TCOL = 512                 # stored C/S table columns (stage-2 lhsT needs 512)

# per-(expert, v-chunk, u-chunk) buckets; expected fill 2048/36 ~ 57 (sigma
# ~7.3), padded to 128 so each (slot, bucket) build matmul is one 128-wide
# N=128 fp16 instruction that never crosses a PSUM bank boundary.
BPAD = 128                               # entries per bucket
BROW = NBLK * BPAD                       # 768 entries per (expert, v-chunk)
EROW = NBLK * BROW                       # 4608 entries per expert
COLS = EROW // 128                       # 36 gather columns per expert slot
GCOLS = 3 * COLS                         # umod | vm | cv interleaved per partition

F32 = mybir.dt.float32
F16 = mybir.dt.float16
I32 = mybir.dt.int32
AOT = mybir.AluOpType
REV = N - HALF             # 382 mirrored columns

KERNEL_TRACE = False       # test harness can flip this to profile
LAST_RESULT = None

_NC = None


def _build():
    nc = bacc.Bacc(trn_type="TRN2")

    cls4 = nc.dram_tensor("cls4", [BPC, N], F32, kind="ExternalInput")
    wr = nc.dram_tensor("wr", [E, N], F32, kind="ExternalInput")
    br = nc.dram_tensor("br", [E], F32, kind="ExternalInput")
    ft = nc.dram_tensor("ft", [E, 3 * EROW], F32, kind="ExternalInput")
    bases = nc.dram_tensor("bases", [E, 1], F32, kind="ExternalInput")
    jm = nc.dram_tensor("jm", [128, 128], F16, kind="ExternalInput")
    ct = nc.dram_tensor("ct", [N, TCOL], F16, kind="ExternalInput")
    st = nc.dram_tensor("st", [N, TCOL], F16, kind="ExternalInput")
    out4 = nc.dram_tensor("out4", [BPC, N, N], F32, kind="ExternalOutput")

    with tile.TileContext(nc) as tc:
        with (
            tc.tile_pool(name="const", bufs=1) as cpool,
            tc.tile_pool(name="tables", bufs=1) as tpool,
            tc.tile_pool(name="routing", bufs=1) as rpool,
            tc.tile_pool(name="gath", bufs=1) as gpool,
            tc.tile_pool(name="build", bufs=20) as bpool,
            tc.tile_pool(name="mt", bufs=2) as mtpool,
            tc.tile_pool(name="pq", bufs=1) as pqpool,
            tc.tile_pool(name="outp", bufs=3) as opool,
            tc.tile_pool(name="psA", bufs=2, space="PSUM") as psA,
            tc.tile_pool(name="psA1", bufs=3, space="PSUM") as psA1,
            tc.tile_pool(name="psB", bufs=1, space="PSUM") as psB,
            tc.tile_pool(name="mir", bufs=4) as mirpool,
            tc.tile_pool(name="t1p", bufs=2) as t1pool,
        ):
            ident = cpool.tile([128, 128], F32)
            make_identity(nc, ident[:])
            ones1 = cpool.tile([1, 128], F32)
            nc.vector.memset(ones1[:], 1.0)
            ones14 = cpool.tile([1, BPC], F32)
            nc.vector.memset(ones14[:], 1.0)
            i128 = cpool.tile([128, 128], I32)
            nc.gpsimd.iota(i128[:], pattern=[[1, 128]], base=0, channel_multiplier=0)
            i128h = cpool.tile([128, 128], F16)
            nc.vector.tensor_copy(i128h[:], i128[:])
            io72 = cpool.tile([128, 1], I32)
            nc.gpsimd.iota(io72[:], pattern=[[0, 1]], base=0, channel_multiplier=GCOLS)
            io72f = cpool.tile([128, 1], F32)
            nc.vector.tensor_copy(io72f[:], io72[:])

            br_sb = rpool.tile([1, E], F32)
            nc.sync.dma_start(out=br_sb[:], in_=br[None, :])
            bases_sb = rpool.tile([E, 1], F32)
            nc.sync.dma_start(out=bases_sb[:], in_=bases[:])
            jJ = cpool.tile([128, 128], F16)
            nc.sync.dma_start(out=jJ[:], in_=jm[:])

            # ---- router: logits = cls4 @ Wr.T + br ----
            comb = rpool.tile([E + BPC, N], F32)
            nc.sync.dma_start(out=comb[0:BPC, :], in_=cls4[:])
            nc.sync.dma_start(out=comb[BPC : BPC + E, :], in_=wr[:])
            combt = rpool.tile([128, NBLK * (E + BPC)], F32)
            for j in range(NBLK):
                tp = psB.tile([128, E + BPC], F32, tag="small")
                nc.tensor.transpose(
                    tp[:],
                    comb[0 : E + BPC, 128 * j : 128 * (j + 1)],
                    ident[0 : E + BPC, 0 : E + BPC],
                )
                nc.scalar.copy(combt[:, (E + BPC) * j : (E + BPC) * (j + 1)], tp[:])
            lg_ps = psB.tile([BPC, E], F32, tag="small")
            for j in range(NBLK):
                base = (E + BPC) * j
                nc.tensor.matmul(
                    lg_ps[:],
                    lhsT=combt[:, base : base + BPC],
                    rhs=combt[:, base + BPC : base + BPC + E],
                    start=(j == 0),
                    stop=False,
                )
            nc.tensor.matmul(
                lg_ps[:], lhsT=ones14[:], rhs=br_sb[:], start=False, stop=True
            )
            logits = rpool.tile([BPC, E], F32)
            nc.vector.tensor_copy(logits[:], lg_ps[:])

            # ---- top-2, renormalized weights, one-hot selectors ----
            max8 = rpool.tile([BPC, 8], F32)
            nc.vector.max(out=max8[:], in_=logits[:])
            l0 = max8[:, 0:1]
            l1 = max8[:, 1:2]
            d = rpool.tile([BPC, 1], F32)
            nc.vector.tensor_sub(d[:], l1, l0)  # l1 - l0
            dT_ps = psB.tile([1, BPC], F32, tag="small")
            nc.tensor.transpose(dT_ps[:], d[:], ident[0:BPC, 0:BPC])
            dT = rpool.tile([1, BPC], F32)
            nc.vector.tensor_copy(dT[:], dT_ps[:])
            w1T = rpool.tile([1, BPC], F32)
            nc.scalar.activation(w1T[:], dT[:], mybir.ActivationFunctionType.Sigmoid)
            w0T = rpool.tile([1, BPC], F32)
            nc.scalar.activation(
                w0T[:], dT[:], mybir.ActivationFunctionType.Sigmoid, scale=-1.0
            )
            oh1 = rpool.tile([BPC, E], F32)
            oh2 = rpool.tile([BPC, E], F32)
            nc.vector.tensor_scalar(oh1[:], logits[:], l0, None, op0=AOT.is_equal)
            nc.vector.tensor_scalar(oh2[:], logits[:], l1, None, op0=AOT.is_equal)
            selT = []
            for srcap in (oh1, oh2):
                sp = psB.tile([E, BPC], F32, tag="small")
                nc.tensor.transpose(sp[:], srcap[:], ident[0:BPC, 0:BPC])
                sbt = rpool.tile([E, BPC], F32, tag=f"selT{len(selT)}")
                nc.vector.tensor_copy(sbt[:], sp[:])
                selT.append(sbt)
            o1T, o2T = selT

            # per-sample scalar rows [1, BPC]: expert table offsets
            eT = []
            for oT in (o1T, o2T):
                ep = psB.tile([1, BPC], F32, tag="small")
                nc.tensor.matmul(
                    ep[:], lhsT=bases_sb[:], rhs=oT[:], start=True, stop=True
                )
                es = rpool.tile([1, BPC], F32, tag=f"eT{len(eT)}")
                nc.vector.tensor_copy(es[:], ep[:])
                eT.append(es)

            # broadcast each scalar row to all 128 partitions: [128, BPC]
            bc = []
            for rowap in (eT[0], eT[1], w0T, w1T):
                bp = psB.tile([128, BPC], F32, tag="small")
                nc.tensor.matmul(
                    bp[:], lhsT=ones1[:], rhs=rowap[:], start=True, stop=True
                )
                bs = rpool.tile([128, BPC], F32, tag=f"bc{len(bc)}")
                nc.vector.tensor_copy(bs[:], bp[:])
                bc.append(bs)
            ebc = bc[0:2]    # expert base offsets per slot
            wbc = bc[2:4]    # expert weights per slot

            # ---- C/S table loads AFTER the routing-phase emission so the
            # small router DMAs aren't queued behind bulk on the sync FIFO
            ct_sb = tpool.tile([128, NBLK * TCOL], F16, tag="ct")
            st_sb = tpool.tile([128, NBLK * TCOL], F16, tag="st")
            for j in range(NBLK):
                nc.sync.dma_start(
                    out=ct_sb[:, TCOL * j : TCOL * (j + 1)],
                    in_=ct[128 * j : 128 * (j + 1), :],
                )
                nc.sync.dma_start(
                    out=st_sb[:, TCOL * j : TCOL * (j + 1)],
                    in_=st[128 * j : 128 * (j + 1), :],
                )

            def emit_d(b, di, mc):
                dps = psA.tile([128, N], F32, tag="mm")
                for c0, cw in ((0, 512), (512, 256)):
                    nc.tensor.matmul(
                        dps[:, c0 : c0 + cw],
                        lhsT=jJ[:],
                        rhs=mc[:, c0 : c0 + cw],
                        start=True, stop=True,
                    )
                ob = opool.tile([128, N], F32, tag="ob")
                nc.scalar.copy(ob[:], dps[:])
                nc.scalar.dma_start(
                    out=out4[:][b][128 * (4 + di) : 128 * (5 + di), :], in_=ob[:]
                )

            # ---- gather ALL samples' (u, vm, coeff) entry tables upfront ----
            # one fused indirect DMA per (sample, slot): per-partition run of
            # COLS u-values, COLS v-mod values, COLS coefficients
            allg = []
            for b in range(BPC):
                per_slot = []
                for slot in range(2):
                    offf = gpool.tile([128, 1], F32, tag="offf")
                    nc.vector.tensor_add(offf[:], ebc[slot][:, b : b + 1], io72f[:])
                    offs = gpool.tile([128, 1], I32, tag="offs")
                    nc.vector.tensor_copy(offs[:], offf[:])
                    gg = gpool.tile([128, GCOLS], F32, tag=f"gg{b}_{slot}")
                    nc.gpsimd.indirect_dma_start(
                        out=gg[:],
                        out_offset=None,
                        in_=ft[:],
                        in_offset=IndirectOffsetOnAxis(ap=offs[:], axis=1),
                    )
                    gcw = gpool.tile([128, COLS], F32, tag=f"gcw{b}_{slot}")
                    nc.vector.tensor_scalar(
                        gcw[:], gg[:, 2 * COLS : 3 * COLS],
                        wbc[slot][:, b : b + 1], None, op0=AOT.mult,
                    )
                    per_slot.append((gg, gcw))
                allg.append(per_slot)

            for b in range(BPC):
                per_slot = allg[b]
                # ---- build MT (transposed mask) chunk by chunk on PE ----
                mt_sb = mtpool.tile([128, NBLK * N], F16, tag="mt")
                for j in range(NBLK):
                    mtps = psA.tile([128, N], F32, tag="mm")
                    for ub in range(NBLK):
                        for slot in range(2):
                            gg, gcw = per_slot[slot]
                            col = NBLK * j + ub
                            voh = bpool.tile([128, 128], F16, tag="voh")
                            nc.vector.tensor_scalar(
                                voh[:], i128h[:],
                                gg[:, COLS + col : COLS + col + 1],
                                None, op0=AOT.is_equal,
                            )
                            rhsb = bpool.tile([128, 128], F16, tag="rhsb")
                            nc.vector.tensor_scalar(
                                rhsb[:], i128h[:],
                                gg[:, col : col + 1],
                                gcw[:, col : col + 1],
                                op0=AOT.is_equal, op1=AOT.mult,
                            )
                            nc.tensor.matmul(
                                mtps[:, 128 * ub : 128 * (ub + 1)],
                                lhsT=voh[:],
                                rhs=rhsb[:],
                                start=(slot == 0),
                                stop=(slot == 1),
                            )
                    nc.scalar.copy(mt_sb[:, N * j : N * (j + 1)], mtps[:])

                # ---- stage 1: P = 300*(M @ C), Qn = -300*(M @ S), cols [0,386)
                p_sb = pqpool.tile([128, NBLK * HALF], F16, tag="p")
                q_sb = pqpool.tile([128, NBLK * HALF], F16, tag="q")
                for i in range(NBLK):
                    pps = psA1.tile([128, HALF], F32, tag="mm1")
                    qps = psA1.tile([128, HALF], F32, tag="mm1")
                    for k in range(NBLK):
                        lhs = mt_sb[:, N * k + 128 * i : N * k + 128 * (i + 1)]
                        nc.tensor.matmul(
                            pps[:], lhsT=lhs, rhs=ct_sb[:, TCOL * k : TCOL * k + HALF],
                            start=(k == 0), stop=(k == NBLK - 1),
                        )
                        nc.tensor.matmul(
                            qps[:], lhsT=lhs, rhs=st_sb[:, TCOL * k : TCOL * k + HALF],
                            start=(k == 0), stop=(k == NBLK - 1),
                        )
                    nc.scalar.mul(p_sb[:, HALF * i : HALF * (i + 1)], pps[:], ALPHA)
                    nc.scalar.mul(q_sb[:, HALF * i : HALF * (i + 1)], qps[:], -ALPHA)

                # ---- stage 2: rows 0..511 via T1/T2 on 386 cols; columns
                # 386..767 by symmetry; rows 512..767 mirrored via emit_d.
                mirs = []
                for i in range(4):
                    t1 = psA1.tile([128, HALF], F32, tag="mm1")
                    t2 = psA1.tile([128, HALF], F32, tag="mm1")
                    for dst, tbl, srcm in ((t1, ct_sb, p_sb), (t2, st_sb, q_sb)):
                        for k in range(NBLK):
                            nc.tensor.matmul(
                                dst[:],
                                lhsT=tbl[:, TCOL * k + 128 * i : TCOL * k + 128 * (i + 1)],
                                rhs=srcm[:, HALF * k : HALF * (k + 1)],
                                start=(k == 0),
                                stop=(k == NBLK - 1),
                            )
                    t1s = t1pool.tile([128, HALF], F16, tag="t1")
                    nc.scalar.copy(t1s[:], t1[:])
                    ob = opool.tile([128, N], F32, tag="ob")
                    nc.vector.tensor_tensor(ob[:, 0:HALF], t1s[:], t2[:], op=AOT.add)
                    if i < 2:
                        m = mirpool.tile([128, N], F16, tag=f"mc{i}")
                        dcap = m[:, 0:HALF]
                    else:
                        dtl = t1pool.tile([128, HALF], F16, tag="dt")
                        dcap = dtl[:]
                    nc.vector.tensor_tensor(dcap, t1s[:], t2[:], op=AOT.subtract)
                    # out[:, 386:768] = (T1-T2)[:, 382..1]
                    nc.scalar.copy(ob[:, HALF:N], dcap[:, REV:0:-1])
                    if i < 2:
                        # m right half: rev (T1+T2) cols 382..1
                        nc.scalar.copy(m[:, HALF:N], ob[:, REV:0:-1])
                        mirs.append(m)
                    elif i == 2:
                        # mc1 row 0 = block-2 row 0 (x = 256)
                        nc.scalar.copy(mirs[1][0:1, 0:HALF], dcap[0:1, :])
                        nc.scalar.copy(mirs[1][0:1, HALF:N], ob[0:1, REV:0:-1])
                    nc.scalar.dma_start(
                        out=out4[:][b][128 * i : 128 * (i + 1), :], in_=ob[:]
                    )
                    if i == 1:
                        # mc0 row 0 = block-1 row 0 (x = 128)
                        nc.scalar.copy(mirs[0][0:1, :], m[0:1, :])
                emit_d(b, 0, mirs[1])
                emit_d(b, 1, mirs[0])

    nc.compile()
    return nc


def _get_nc():
    global _NC
    if _NC is None:
        _NC = _build()
    return _NC


def _host_tables():
    a = np.arange(N, dtype=np.int64)
    ang = (2.0 * np.pi / N) * ((a[:, None] * a[None, :]) % N)
    ctv = (np.cos(ang) / N).astype(np.float16)[:, 0:TCOL]
    stv = (np.sin(ang) / N).astype(np.float16)[:, 0:TCOL]
    return np.ascontiguousarray(ctv), np.ascontiguousarray(stv)


def _host_entry_tables(list_indices, coeff):
    """Bucket each expert's (u, v, coeff) entries by (v-chunk, u-half), pad
    buckets to PAD, lay out partition-major, and interleave u/vm/cv per
    partition so one indirect DMA fetches all three."""
    li = list_indices.astype(np.int64)
    uu = li // N
    vv = li % N
    u2 = np.full((E, EROW), -9.0, np.float32)
    vm2 = np.full((E, EROW), -9.0, np.float32)
    cv2 = np.zeros((E, EROW), np.float32)
    for e in range(E):
        for j in range(NBLK):
            selj = vv[e] // 128 == j
            for ub in range(NBLK):
                sel = np.where(selj & (uu[e] // 128 == ub))[0]
                cnt = len(sel)
                assert cnt <= BPAD, f"bucket overflow: e{e} j{j} ub{ub}: {cnt}"
                base = BROW * j + BPAD * ub
                u2[e, base : base + cnt] = uu[e, sel] - 128 * ub
                vm2[e, base : base + cnt] = vv[e, sel] - 128 * j
                cv2[e, base : base + cnt] = coeff[e, sel]
    # fused layout: ftab[e, p*GCOLS + t*COLS + g] = arr_t[e, 128*g + p]
    ftab = np.zeros((E, 3 * EROW), np.float32)
    p_ix = np.arange(128)[:, None]
    g_ix = np.arange(COLS)[None, :]
    src = (128 * g_ix + p_ix).reshape(-1)          # [128*COLS] entry index
    for t, arr in enumerate((u2, vm2, cv2)):
        dst = (p_ix * GCOLS + t * COLS + g_ix).reshape(-1)
        ftab[:, dst] = arr[:, src]
    return ftab


def kernel(cls_token, W_router, b_router, coeff, list_indices):
    global LAST_RESULT
    cls_token = np.asarray(cls_token)
    W_router = np.asarray(W_router)
    b_router = np.asarray(b_router)
    coeff = np.asarray(coeff)
    list_indices = np.asarray(list_indices)
    assert cls_token.shape == (B, N) and coeff.shape == (E, NF)
    nc = _get_nc()
    ctv, stv = _host_tables()
    ftv = _host_entry_tables(list_indices, coeff)
    basesv = (np.arange(E, dtype=np.float32) * (3 * EROW)).reshape(E, 1)
    jmv = np.zeros((128, 128), np.float16)
    for m_ in range(128):
        jmv[(128 - m_) % 128, m_] = 1.0
    wrr = np.ascontiguousarray(W_router, dtype=np.float32)
    brr = np.ascontiguousarray(b_router, dtype=np.float32)
    in_maps = []
    for c in range(NCORES):
        in_maps.append(
            {
                "cls4": np.ascontiguousarray(
                    cls_token[BPC * c : BPC * (c + 1)], dtype=np.float32
                ),
                "wr": wrr,
                "br": brr,
                "ft": ftv,
                "bases": basesv,
                "jm": jmv,
                "ct": ctv,
                "st": stv,
            }
        )
    res = run_bass_kernel_spmd(
        nc, in_maps, core_ids=list(range(NCORES)), trace=KERNEL_TRACE
    )
    LAST_RESULT = res
    out = np.concatenate([res.results[c]["out4"] for c in range(NCORES)], axis=0)
    return out
